# revision 23
# baseline (speedup 1.0000x reference)
"""Trainium2 Bass kernel for a 2-layer LSTM (batch 8192, seq 128, in 32, hidden 64)
with a final linear head producing one logit per batch element.

Strategy: pure data parallel over 8 NeuronCores (1024 batch each), weights
replicated.  The input projection is folded into the recurrent step (no
[B,T,4H] materialization) so HBM traffic is ~one read of x.

The ACT (scalar) engine is the bottleneck: the LSTM needs 5 transcendental
values per hidden unit per layer-step (4 gate sigmoids + tanh(c); tanh(g) is
computed as 2*sigmoid(2g)-1 with gate weights pre-scaled by 2), and ACT
streams 1 column/cycle regardless of dtype.  So ACT instructions are made as
wide as the recurrence allows, and everything else is kept off its critical
path:

- 2 pipeline chains of 512 batch (x2 partition halves of 256 columns),
  half-phase offset: chain B executes its layer-1 stage while chain A
  executes layer 0, so each chain's recurrence latency hides under the other
  chain's ACT work.
- Per chain-timestep ACT runs only 3 instructions: sigmoid over all 4 L0
  gates [128,1024], one merged tanh over [c1(t-1) | c0(t)] [128,512], and
  sigmoid over all 4 L1 gates [128,1024].
- L1's bias (forget gate +1) is accumulated into PSUM by one rank-1 ones
  matmul so the L1 sigmoid needs no per-block bias split.
- The f*c_prev product of layer 1 runs on the GpSimd (Pool) engine to keep
  DVE under the ACT shadow.  Matmuls use float32r (full-rate fp32).
"""

import numpy as np

INPUT = 32
HIDDEN = 64
BATCH = 8192
SEQ = 128
NCORES = 8
BCORE = BATCH // NCORES      # 1024
NCH = 2                      # pipeline chains per core
BHC = BCORE // (2 * NCH)     # 256 columns per chain (x2 partition halves)
D1 = INPUT + 1               # x rows + ones row

_CACHE = {}


def _build_module(b1_const):
    """b1_const: length-4 list of per-gate constant L1 biases (device L1 gate
    order [i, g, o, f]); None entries mean a non-constant bias vector (falls
    back to per-gate rank-1 matmuls from b1st)."""
    import concourse.bacc as bacc
    import concourse.mybir as mybir
    import concourse.tile as tile

    F32 = mybir.dt.float32
    F32R = mybir.dt.float32r
    AF = mybir.ActivationFunctionType
    MUL = mybir.AluOpType.mult
    ADD = mybir.AluOpType.add
    SUB = mybir.AluOpType.subtract

    nc = bacc.Bacc()
    # L0 gate blocks in P0: [i, g, f, o];  L1 gate blocks in P1: [i, g, o, f].
    xT = nc.dram_tensor("xT", [SEQ, NCH, 2 * D1, BHC], F32R, kind="ExternalInput")
    wx0 = nc.dram_tensor("wx0", [4, 2 * D1, 128], F32R, kind="ExternalInput")
    wh0 = nc.dram_tensor("wh0", [4, 128, 128], F32R, kind="ExternalInput")
    w1a = nc.dram_tensor("w1a", [4, 128, 128], F32R, kind="ExternalInput")
    w1b = nc.dram_tensor("w1b", [4, 128, 128], F32R, kind="ExternalInput")
    b1st = nc.dram_tensor("b1st", [4, 1, 128], F32R, kind="ExternalInput")
    bpat = nc.dram_tensor("bpat", [1, 4 * BHC], F32R, kind="ExternalInput")
    ones = nc.dram_tensor("ones", [1, 4 * BHC], F32R, kind="ExternalInput")
    fcw = nc.dram_tensor("fcw", [128, 2], F32R, kind="ExternalInput")
    fcb = nc.dram_tensor("fcb", [2, 1], F32, kind="ExternalInput")
    out = nc.dram_tensor("out", [NCH, 2, BHC], F32, kind="ExternalOutput")

    with tile.TileContext(nc) as tc:
        with (
            tc.tile_pool(name="wp", bufs=1) as wp,
            tc.tile_pool(name="sb", bufs=3) as sb,
            tc.tile_pool(name="ps", bufs=1, space="PSUM") as ps,
        ):
            twx = [wp.tile([2 * D1, 128], F32R, name=f"twx{g}", tag=f"twx{g}") for g in range(4)]
            twh = [wp.tile([128, 128], F32R, name=f"twh{g}", tag=f"twh{g}") for g in range(4)]
            t1a = [wp.tile([128, 128], F32R, name=f"t1a{g}", tag=f"t1a{g}") for g in range(4)]
            t1b = [wp.tile([128, 128], F32R, name=f"t1b{g}", tag=f"t1b{g}") for g in range(4)]
            tb1 = [wp.tile([1, 128], F32R, name=f"tb1{g}", tag=f"tb1{g}") for g in range(4)]
            tbp = wp.tile([1, 4 * BHC], F32R, name="tbp")
            tone = wp.tile([1, 4 * BHC], F32R, name="tone")
            tfcw = wp.tile([128, 2], F32R, name="tfcw")
            tfcb = wp.tile([2, 1], F32, name="tfcb")
            for g in range(4):
                nc.sync.dma_start(twx[g][:, :], wx0[g, :, :])
                nc.sync.dma_start(twh[g][:, :], wh0[g, :, :])
                nc.sync.dma_start(t1a[g][:, :], w1a[g, :, :])
                nc.sync.dma_start(t1b[g][:, :], w1b[g, :, :])
                nc.sync.dma_start(tb1[g][:, :], b1st[g, :, :])
            nc.sync.dma_start(tbp[:, :], bpat[0:1, :])
            nc.sync.dma_start(tone[:, :], ones[0:1, :])
            nc.sync.dma_start(tfcw[:, :], fcw[:, :])
            nc.sync.dma_start(tfcb[:, :], fcb[:, :])

            # Per-chain state (python handles to live tiles)
            st = [dict(h1=None, h2=None, ga=None, sg=None, th=None,
                       cc=None, ccp=None) for _ in range(NCH)]

            def emit_whwx(ch, t):
                """PE fill of P0(ch, t) (+ xt DMA).  wh needs h1(ch, t-1)."""
                s = st[ch]
                C = f"c{ch}_"
                xt = sb.tile([2 * D1, BHC], F32R, name=f"{C}xt{t}", tag=f"{C}xt", bufs=4)
                nc.sync.dma_start(xt[:, :], xT[t, ch, :, :])
                P = ps.tile([128, 4 * BHC], F32, name=f"{C}P0_{t}", tag=f"{C}P0", bufs=1)
                for g in range(4):
                    blk = slice(g * BHC, (g + 1) * BHC)
                    if t > 0:
                        nc.tensor.matmul(P[:, blk], twh[g][:, :], s["h1"][:, :],
                                         start=True, stop=False)
                    nc.tensor.matmul(P[:, blk], twx[g][:, :], xt[:, :],
                                     start=(t == 0), stop=True)
                s["P0"] = P

            # Schedule: half-period k runs (X = chain k%2, tX = k//2,
            # tY = (k-1)//2, Y = other):
            #   PE:   w1a/w1b/bias of L1_X(tX-1)
            #   ACT:  sigmoid L0_X(tX)          -> sg_X
            #   Pool: fc0_X(tX); DVE: ig0_X(tX)
            #   ACT:  tanh_Y(tY) over [c1_Y(tY-1) | c0_Y(tY)]
            #   DVE:  h1_Y(tY), h2_Y(tY-1)
            #   PE:   wh/wx fill of P0_Y(tY+1)
            #   DVE:  c0_X(tX)
            #   ACT:  sigmoid L1_X(tX-1)        -> ga_X
            #   DVE:  ig1, fc1, c1_X(tX-1)
            # Each tanh sits a full sigmoid after its DVE producers, and each
            # sigmoid's PE inputs complete a half-period ahead.
            emit_whwx(0, 0)
            emit_whwx(1, 0)
            for k in range(2 * SEQ + 2):
                X = k % 2
                Y = 1 - X
                tX = k // 2
                tY = (k - 1) // 2
                sX, sY = st[X], st[Y]
                CX = f"c{X}_"
                CY = f"c{Y}_"
                tL1 = tX - 1

                # -- PE: P1_X(tL1) --
                if 0 <= tL1 < SEQ:
                    P1 = ps.tile([128, 4 * BHC], F32, name=f"{CX}P1_{tL1}",
                                 tag=f"{CX}P1", bufs=1)
                    for g in range(4):
                        blk = slice(g * BHC, (g + 1) * BHC)
                        nc.tensor.matmul(P1[:, blk], t1a[g][:, :], sX["h1"][:, :],
                                         start=True, stop=False)
                        if tL1 > 0:
                            nc.tensor.matmul(P1[:, blk], t1b[g][:, :], sX["h2"][:, :],
                                             start=False, stop=False)
                        nc.tensor.matmul(P1[:, blk], tb1[g][:, :], tone[:, 0:BHC],
                                         start=False, stop=True)

                # -- ACT: sigma L0_X(tX) over [i, g, f, o] --
                if tX < SEQ:
                    sg = sb.tile([128, 4 * BHC], F32, name=f"{CX}sg{tX}",
                                 tag=f"{CX}sg", bufs=2)
                    nc.scalar.activation(sg[:, :], sX["P0"][:, :], AF.Sigmoid)
                    sX["sg"] = sg
                    # rotate cc: cc_X(tX) = [c1_X(tX-1) | c0_X(tX)]
                    sX["ccp"] = sX["cc"]
                    sX["cc"] = sb.tile([128, 2 * BHC], F32, name=f"{CX}cc{tX}",
                                       tag=f"{CX}cc", bufs=3)
                    # fc0 on Pool (off DVE's critical chain)
                    if tX > 0:
                        fc0 = sb.tile([128, BHC], F32, name=f"{CX}fc0_{tX}",
                                      tag=f"{CX}fc0", bufs=2)
                        nc.gpsimd.tensor_tensor(fc0[:, :], sg[:, 2 * BHC:3 * BHC],
                                                sX["ccp"][:, BHC:2 * BHC], MUL)
                    ig0 = sb.tile([128, BHC], F32, name=f"{CX}ig0_{tX}",
                                  tag=f"{CX}ig0", bufs=2)
                    nc.vector.scalar_tensor_tensor(
                        ig0[:, :], sg[:, BHC:2 * BHC], 0.5, sg[:, 0:BHC], SUB, MUL)
                elif tX == SEQ:
                    # epilogue halves: the final c1_X(SEQ-1) still needs a home
                    sX["ccp"] = sX["cc"]
                    sX["cc"] = sb.tile([128, 2 * BHC], F32, name=f"{CX}cc{tX}",
                                       tag=f"{CX}cc", bufs=3)

                # -- ACT: tanh_Y(tY);  DVE: h1_Y(tY), h2_Y(tY-1) --
                if 0 <= tY < SEQ:
                    th = sb.tile([128, 2 * BHC], F32, name=f"{CY}th{tY}",
                                 tag=f"{CY}th", bufs=2)
                    if tY == 0:
                        nc.scalar.activation(th[:, BHC:2 * BHC],
                                             sY["cc"][:, BHC:2 * BHC], AF.Tanh)
                    else:
                        nc.scalar.activation(th[:, :], sY["cc"][:, :], AF.Tanh)
                    h1 = sb.tile([128, BHC], F32R, name=f"{CY}h1_{tY}",
                                 tag=f"{CY}h1", bufs=2)
                    nc.vector.tensor_tensor(h1[:, :], sY["sg"][:, 3 * BHC:4 * BHC],
                                            th[:, BHC:2 * BHC], MUL)
                    sY["h1"] = h1
                    if tY > 0:
                        h2 = sb.tile([128, BHC], F32R, name=f"{CY}h2_{tY-1}",
                                     tag=f"{CY}h2", bufs=2)
                        nc.vector.tensor_tensor(h2[:, :], sY["ga"][:, 2 * BHC:3 * BHC],
                                                th[:, 0:BHC], MUL)
                        sY["h2"] = h2
                    # -- PE: fill P0_Y(tY+1) --
                    if tY + 1 < SEQ:
                        emit_whwx(Y, tY + 1)

                # -- DVE: c0_X(tX) --
                if tX < SEQ:
                    if tX == 0:
                        nc.vector.tensor_scalar_mul(sX["cc"][:, BHC:2 * BHC],
                                                    ig0[:, :], 2.0)
                    else:
                        nc.vector.scalar_tensor_tensor(
                            sX["cc"][:, BHC:2 * BHC], ig0[:, :], 2.0, fc0[:, :],
                            MUL, ADD)

                # -- ACT: sigma L1_X(tL1); DVE: ig1, fc1, c1 --
                if 0 <= tL1 < SEQ:
                    ga = sb.tile([128, 4 * BHC], F32, name=f"{CX}ga{tL1}",
                                 tag=f"{CX}ga", bufs=2)
                    nc.scalar.activation(ga[:, :], P1[:, :], AF.Sigmoid)
                    sX["ga"] = ga
                    ig1 = sb.tile([128, BHC], F32, name=f"{CX}ig1_{tL1}",
                                  tag=f"{CX}ig1", bufs=2)
                    nc.vector.scalar_tensor_tensor(
                        ig1[:, :], ga[:, BHC:2 * BHC], 0.5, ga[:, 0:BHC], SUB, MUL)
                    # c1_X(tL1) -> left half of cc_X(tX)
                    if tL1 == 0:
                        nc.vector.tensor_scalar_mul(sX["cc"][:, 0:BHC], ig1[:, :], 2.0)
                    else:
                        fc1 = sb.tile([128, BHC], F32, name=f"{CX}fc1_{tL1}",
                                      tag=f"{CX}fc1", bufs=2)
                        nc.vector.tensor_tensor(fc1[:, :], ga[:, 3 * BHC:4 * BHC],
                                                sX["ccp"][:, 0:BHC], MUL)
                        nc.vector.scalar_tensor_tensor(
                            sX["cc"][:, 0:BHC], ig1[:, :], 2.0, fc1[:, :], MUL, ADD)

            # ---- final h2(SEQ-1) + linear head per chain ----
            for ch in range(NCH):
                s = st[ch]
                C = f"c{ch}_"
                thf = sb.tile([128, BHC], F32, name=f"{C}thf", tag=f"{C}th", bufs=2)
                nc.scalar.activation(thf[:, :], s["cc"][:, 0:BHC], AF.Tanh)
                h2 = sb.tile([128, BHC], F32R, name=f"{C}h2f", tag=f"{C}h2", bufs=2)
                nc.vector.tensor_tensor(h2[:, :], s["ga"][:, 2 * BHC:3 * BHC],
                                        thf[:, :], MUL)
                Pf = ps.tile([2, BHC], F32, name=f"Pf{ch}", tag=f"{C}P0")
                nc.tensor.matmul(Pf[:, :], tfcw[:, :], h2[:, :], start=True, stop=True)
                ob = sb.tile([2, BHC], F32, name=f"ob{ch}")
                nc.scalar.activation(ob[:, :], Pf[:, :], AF.Identity, bias=tfcb[:, 0:1])
                nc.sync.dma_start(out[ch, :, :], ob[:, :])

    nc.compile()
    return nc


def _prep_weights(w_ih0, w_hh0, b_ih0, b_hh0, w_ih1, w_hh1, b_ih1, b_hh1, fc_w, fc_b):
    """Host-side packing.  Device gate order: L0 [i, g, f, o], L1 [i, g, o, f]
    (PyTorch order is i, f, g, o)."""
    H = HIDDEN
    GATES0 = [0, 2, 1, 3]       # device k -> pytorch gate for L0 [i, g, f, o]
    GATES1 = [0, 2, 3, 1]       # device k -> pytorch gate for L1 [i, g, o, f]
    b0 = (b_ih0 + b_hh0).reshape(4, H)
    b1 = (b_ih1 + b_hh1).reshape(4, H)
    wi0 = w_ih0.reshape(4, H, INPUT)
    wh0_ = w_hh0.reshape(4, H, H)
    wi1 = w_ih1.reshape(4, H, H)
    wh1_ = w_hh1.reshape(4, H, H)

    wx0 = np.zeros((4, 2 * D1, 128), np.float32)
    wh0 = np.zeros((4, 128, 128), np.float32)
    w1a = np.zeros((4, 128, 128), np.float32)
    w1b = np.zeros((4, 128, 128), np.float32)
    b1st = np.zeros((4, 1, 128), np.float32)
    b1c = [None] * 4
    for k, gi in enumerate(GATES0):
        sc = 2.0 if gi == 2 else 1.0      # pytorch gate 2 = g: pre-scale x2
        wt = sc * wi0[gi].T               # [INPUT, H]
        wx0[k, :INPUT, 0:H] = wt
        wx0[k, INPUT, 0:H] = sc * b0[gi]
        wx0[k, D1:D1 + INPUT, H:2 * H] = wt
        wx0[k, D1 + INPUT, H:2 * H] = sc * b0[gi]
        wh0[k, 0:H, 0:H] = sc * wh0_[gi].T
        wh0[k, H:2 * H, H:2 * H] = sc * wh0_[gi].T
    for k, gi in enumerate(GATES1):
        sc = 2.0 if gi == 2 else 1.0
        w1a[k, 0:H, 0:H] = sc * wi1[gi].T
        w1a[k, H:2 * H, H:2 * H] = sc * wi1[gi].T
        w1b[k, 0:H, 0:H] = sc * wh1_[gi].T
        w1b[k, H:2 * H, H:2 * H] = sc * wh1_[gi].T
        b1st[k, 0, 0:H] = sc * b1[gi]
        b1st[k, 0, H:2 * H] = sc * b1[gi]
        if np.all(b1[gi] == b1[gi][0]):
            b1c[k] = float(sc * b1[gi][0])

    fcw = np.zeros((128, 2), np.float32)
    fcw[0:H, 0] = fc_w[0]
    fcw[H:2 * H, 1] = fc_w[0]
    fcb = np.full((2, 1), np.float32(fc_b[0]), np.float32)
    return wx0, wh0, w1a, w1b, b1st, b1c, fcw, fcb


def run_full(x, w_ih0, w_hh0, b_ih0, b_hh0, w_ih1, w_hh1, b_ih1, b_hh1, fc_w, fc_b,
             trace=False):
    """Run the full problem on 8 cores; returns (output [BATCH], BassKernelResults)."""
    from concourse.bass_utils import run_bass_kernel_spmd

    x = np.asarray(x, np.float32)
    args = [np.asarray(a, np.float32) for a in
            (w_ih0, w_hh0, b_ih0, b_hh0, w_ih1, w_hh1, b_ih1, b_hh1, fc_w, fc_b)]
    wx0, wh0, w1a, w1b, b1st, b1c, fcw, fcb = _prep_weights(*args)

    b1_const = b1c if all(v is not None for v in b1c) else None
    key = ("const", tuple(b1c)) if b1_const is not None else ("vec",)
    if key not in _CACHE:
        _CACHE.clear()
        _CACHE[key] = _build_module(b1_const)
    nc = _CACHE[key]

    bpat = np.zeros((1, 4 * BHC), np.float32)
    if b1_const is not None:
        for g in range(4):
            bpat[0, g * BHC:(g + 1) * BHC] = b1_const[g]

    in_maps = []
    for c in range(NCORES):
        xs = x[c * BCORE:(c + 1) * BCORE]                  # [BCORE, SEQ, INPUT]
        xT = np.empty((SEQ, NCH, 2 * D1, BHC), np.float32)
        for ch in range(NCH):
            a0 = ch * BHC
            b0_ = BCORE // 2 + ch * BHC
            xT[:, ch, :INPUT, :] = xs[a0:a0 + BHC].transpose(1, 2, 0)
            xT[:, ch, INPUT, :] = 1.0
            xT[:, ch, D1:D1 + INPUT, :] = xs[b0_:b0_ + BHC].transpose(1, 2, 0)
            xT[:, ch, D1 + INPUT, :] = 1.0
        in_maps.append({
            "xT": xT, "wx0": wx0, "wh0": wh0, "w1a": w1a, "w1b": w1b,
            "b1st": b1st, "bpat": bpat, "ones": np.ones((1, 4 * BHC), np.float32),
            "fcw": fcw, "fcb": fcb,
        })

    res = run_bass_kernel_spmd(nc, in_maps, core_ids=list(range(NCORES)), trace=trace)
    outs = []
    for r in res.results:
        o = r["out"]                        # [NCH, 2, BHC]: (chain, half, col)
        # per-core batch order: [ch0 halfA, ch1 halfA, ch0 halfB, ch1 halfB]
        outs.append(o.transpose(1, 0, 2).reshape(BCORE))
    return np.concatenate(outs, axis=0).astype(np.float32), res


def kernel(x, w_ih0, w_hh0, b_ih0, b_hh0, w_ih1, w_hh1, b_ih1, b_hh1, fc_w, fc_b):
    out, _ = run_full(x, w_ih0, w_hh0, b_ih0, b_hh0,
                      w_ih1, w_hh1, b_ih1, b_hh1, fc_w, fc_b)
    return out


# revision 26
# speedup vs baseline: 1.2163x; 1.2163x over previous
"""Trainium2 Bass kernel for a 2-layer LSTM (batch 8192, seq 128, in 32, hidden 64)
with a final linear head producing one logit per batch element.

Strategy: pure data parallel over 8 NeuronCores (1024 batch each), weights
replicated.  The input projection is folded into the recurrent step (no
[B,T,4H] materialization) so HBM traffic is ~one read of x.

The ACT (scalar) engine is the bottleneck: the LSTM needs 5 transcendental
values per hidden unit per layer-step (4 gate sigmoids + tanh(c); tanh(g) is
computed as 2*sigmoid(2g)-1 with gate weights pre-scaled by 2), and ACT
streams 1 column/cycle regardless of dtype.  So ACT instructions are made as
wide as the recurrence allows, and everything else is kept off its critical
path:

- 2 pipeline chains of 512 batch (x2 partition halves of 256 columns),
  half-phase offset: chain B executes its layer-1 stage while chain A
  executes layer 0, so each chain's recurrence latency hides under the other
  chain's ACT work.
- Per chain-timestep ACT runs only 3 instructions: sigmoid over all 4 L0
  gates [128,1024], one merged tanh over [c1(t-1) | c0(t)] [128,512], and
  sigmoid over all 4 L1 gates [128,1024].
- L1's bias (forget gate +1) is accumulated into PSUM by one rank-1 ones
  matmul so the L1 sigmoid needs no per-block bias split.
- The f*c_prev product of layer 1 runs on the GpSimd (Pool) engine to keep
  DVE under the ACT shadow.  Matmuls use float32r (full-rate fp32).
"""

import numpy as np

INPUT = 32
HIDDEN = 64
BATCH = 8192
SEQ = 128
NCORES = 8
BCORE = BATCH // NCORES      # 1024
NCH = 2                      # pipeline chains per core
BHC = BCORE // (2 * NCH)     # 256 columns per chain (x2 partition halves)
D1 = INPUT + 1               # x rows + ones row

_CACHE = {}


def _build_module(b1_const):
    """b1_const: length-4 list of per-gate constant L1 biases (device L1 gate
    order [i, g, o, f]); None entries mean a non-constant bias vector (falls
    back to per-gate rank-1 matmuls from b1st)."""
    import concourse.bacc as bacc
    import concourse.mybir as mybir
    import concourse.tile as tile

    F32 = mybir.dt.float32
    F32R = mybir.dt.float32r
    AF = mybir.ActivationFunctionType
    MUL = mybir.AluOpType.mult
    ADD = mybir.AluOpType.add
    SUB = mybir.AluOpType.subtract

    nc = bacc.Bacc()
    # L0 gate blocks in P0: [i, g, f, o];  L1 gate blocks in P1: [i, g, o, f].
    xT = nc.dram_tensor("xT", [SEQ, NCH, 2 * D1, BHC], F32R, kind="ExternalInput")
    wx0 = nc.dram_tensor("wx0", [4, 2 * D1, 128], F32R, kind="ExternalInput")
    wh0 = nc.dram_tensor("wh0", [4, 128, 128], F32R, kind="ExternalInput")
    w1a = nc.dram_tensor("w1a", [4, 128, 128], F32R, kind="ExternalInput")
    w1b = nc.dram_tensor("w1b", [4, 128, 128], F32R, kind="ExternalInput")
    b1st = nc.dram_tensor("b1st", [4, 1, 128], F32R, kind="ExternalInput")
    bpat = nc.dram_tensor("bpat", [1, 4 * BHC], F32R, kind="ExternalInput")
    ones = nc.dram_tensor("ones", [1, 4 * BHC], F32R, kind="ExternalInput")
    fcw = nc.dram_tensor("fcw", [128, 2], F32R, kind="ExternalInput")
    fcb = nc.dram_tensor("fcb", [2, 1], F32, kind="ExternalInput")
    out = nc.dram_tensor("out", [NCH, 2, BHC], F32, kind="ExternalOutput")

    with tile.TileContext(nc) as tc:
        with (
            tc.tile_pool(name="wp", bufs=1) as wp,
            tc.tile_pool(name="sb", bufs=3) as sb,
            tc.tile_pool(name="ps", bufs=1, space="PSUM") as ps,
        ):
            twx = [wp.tile([2 * D1, 128], F32R, name=f"twx{g}", tag=f"twx{g}") for g in range(4)]
            twh = [wp.tile([128, 128], F32R, name=f"twh{g}", tag=f"twh{g}") for g in range(4)]
            t1a = [wp.tile([128, 128], F32R, name=f"t1a{g}", tag=f"t1a{g}") for g in range(4)]
            t1b = [wp.tile([128, 128], F32R, name=f"t1b{g}", tag=f"t1b{g}") for g in range(4)]
            tb1 = [wp.tile([1, 128], F32R, name=f"tb1{g}", tag=f"tb1{g}") for g in range(4)]
            tbp = wp.tile([1, 4 * BHC], F32R, name="tbp")
            tone = wp.tile([1, 4 * BHC], F32R, name="tone")
            tfcw = wp.tile([128, 2], F32R, name="tfcw")
            tfcb = wp.tile([2, 1], F32, name="tfcb")
            for g in range(4):
                nc.sync.dma_start(twx[g][:, :], wx0[g, :, :])
                nc.sync.dma_start(twh[g][:, :], wh0[g, :, :])
                nc.sync.dma_start(t1a[g][:, :], w1a[g, :, :])
                nc.sync.dma_start(t1b[g][:, :], w1b[g, :, :])
                nc.sync.dma_start(tb1[g][:, :], b1st[g, :, :])
            nc.sync.dma_start(tbp[:, :], bpat[0:1, :])
            nc.sync.dma_start(tone[:, :], ones[0:1, :])
            nc.sync.dma_start(tfcw[:, :], fcw[:, :])
            nc.sync.dma_start(tfcb[:, :], fcb[:, :])

            # Per-chain state (python handles to live tiles)
            st = [dict(h1=None, h2=None, ga=None, sg=None, P0=None,
                       c0=None, c1=None) for _ in range(NCH)]

            def emit_whwx(ch, t):
                """PE fill of P0(ch, t) (+ xt DMA).  wh needs h1(ch, t-1)."""
                s = st[ch]
                C = f"c{ch}_"
                xt = sb.tile([2 * D1, BHC], F32R, name=f"{C}xt{t}", tag=f"{C}xt", bufs=4)
                nc.sync.dma_start(xt[:, :], xT[t, ch, :, :])
                P = ps.tile([128, 4 * BHC], F32, name=f"{C}P0_{t}", tag=f"{C}P0", bufs=1)
                for g in range(4):
                    blk = slice(g * BHC, (g + 1) * BHC)
                    if t > 0:
                        nc.tensor.matmul(P[:, blk], twh[g][:, :], s["h1"][:, :],
                                         start=True, stop=False)
                    nc.tensor.matmul(P[:, blk], twx[g][:, :], xt[:, :],
                                     start=(t == 0), stop=True)
                s["P0"] = P

            # Schedule: half-period k runs (X = chain k%2, Y = other,
            # tX = k//2, tY = (k-1)//2), ACT order
            #   [ tanh0_Y(tY), sigmoid L0_X(tX), tanh1_Y(tY-1), sigmoid L1_X(tX-1) ]
            # tanh0 feeds the critical h1 -> wh -> sigmoid-L0 recurrence with a
            # full half-period of slack; tanh1's c1 producer finished one half
            # earlier; all PE fills complete a half-period ahead of their
            # consumer sigmoids.
            emit_whwx(0, 0)
            emit_whwx(1, 0)
            for k in range(2 * SEQ + 3):
                X = k % 2
                Y = 1 - X
                tX = k // 2
                tY = (k - 1) // 2
                sX, sY = st[X], st[Y]
                CX = f"c{X}_"
                CY = f"c{Y}_"
                tL1 = tX - 1

                # -- PE: P1_X(tL1) --
                if 0 <= tL1 < SEQ:
                    P1 = ps.tile([128, 4 * BHC], F32, name=f"{CX}P1_{tL1}",
                                 tag=f"{CX}P1", bufs=1)
                    for g in range(4):
                        blk = slice(g * BHC, (g + 1) * BHC)
                        nc.tensor.matmul(P1[:, blk], t1a[g][:, :], sX["h1"][:, :],
                                         start=True, stop=False)
                        if tL1 > 0:
                            nc.tensor.matmul(P1[:, blk], t1b[g][:, :], sX["h2"][:, :],
                                             start=False, stop=False)
                        nc.tensor.matmul(P1[:, blk], tb1[g][:, :], tone[:, 0:BHC],
                                         start=False, stop=True)

                # -- ACT: tanh0_Y(tY);  DVE: h1_Y(tY) --
                if 0 <= tY < SEQ:
                    th0 = sb.tile([128, BHC], F32, name=f"{CY}th0_{tY}",
                                  tag=f"{CY}th0", bufs=2)
                    nc.scalar.activation(th0[:, :], sY["c0"][:, :], AF.Tanh)
                    h1 = sb.tile([128, BHC], F32R, name=f"{CY}h1_{tY}",
                                 tag=f"{CY}h1", bufs=2)
                    nc.vector.tensor_tensor(h1[:, :], sY["sg"][:, 3 * BHC:4 * BHC],
                                            th0[:, :], MUL)
                    sY["h1"] = h1

                # -- ACT: sigma L0_X(tX) over [i, g, f, o] --
                if tX < SEQ:
                    sg = sb.tile([128, 4 * BHC], F32, name=f"{CX}sg{tX}",
                                 tag=f"{CX}sg", bufs=2)
                    nc.scalar.activation(sg[:, :], sX["P0"][:, :], AF.Sigmoid)
                    sX["sg"] = sg
                    # fc0 on Pool (off DVE's critical chain)
                    if tX > 0:
                        fc0 = sb.tile([128, BHC], F32, name=f"{CX}fc0_{tX}",
                                      tag=f"{CX}fc0", bufs=2)
                        nc.gpsimd.tensor_tensor(fc0[:, :], sg[:, 2 * BHC:3 * BHC],
                                                sX["c0"][:, :], MUL)
                    ig0 = sb.tile([128, BHC], F32, name=f"{CX}ig0_{tX}",
                                  tag=f"{CX}ig0", bufs=2)
                    nc.vector.scalar_tensor_tensor(
                        ig0[:, :], sg[:, BHC:2 * BHC], 0.5, sg[:, 0:BHC], SUB, MUL)

                # -- ACT: tanh1_Y(tY-1);  DVE: h2_Y(tY-1) --
                if 1 <= tY <= SEQ:
                    th1 = sb.tile([128, BHC], F32, name=f"{CY}th1_{tY-1}",
                                  tag=f"{CY}th1", bufs=2)
                    nc.scalar.activation(th1[:, :], sY["c1"][:, :], AF.Tanh)
                    h2 = sb.tile([128, BHC], F32R, name=f"{CY}h2_{tY-1}",
                                 tag=f"{CY}h2", bufs=2)
                    nc.vector.tensor_tensor(h2[:, :], sY["ga"][:, 2 * BHC:3 * BHC],
                                            th1[:, :], MUL)
                    sY["h2"] = h2

                # -- PE: fill P0_Y(tY+1) --
                if 0 <= tY < SEQ - 1:
                    emit_whwx(Y, tY + 1)

                # -- DVE: c0_X(tX) --
                if tX < SEQ:
                    c0 = sb.tile([128, BHC], F32, name=f"{CX}c0_{tX}",
                                 tag=f"{CX}c0", bufs=2)
                    if tX == 0:
                        nc.vector.tensor_scalar_mul(c0[:, :], ig0[:, :], 2.0)
                    else:
                        nc.vector.scalar_tensor_tensor(
                            c0[:, :], ig0[:, :], 2.0, fc0[:, :], MUL, ADD)
                    sX["c0"] = c0

                # -- ACT: sigma L1_X(tL1); DVE: ig1, fc1, c1 --
                if 0 <= tL1 < SEQ:
                    ga = sb.tile([128, 4 * BHC], F32, name=f"{CX}ga{tL1}",
                                 tag=f"{CX}ga", bufs=2)
                    nc.scalar.activation(ga[:, :], P1[:, :], AF.Sigmoid)
                    sX["ga"] = ga
                    ig1 = sb.tile([128, BHC], F32, name=f"{CX}ig1_{tL1}",
                                  tag=f"{CX}ig1", bufs=2)
                    nc.vector.scalar_tensor_tensor(
                        ig1[:, :], ga[:, BHC:2 * BHC], 0.5, ga[:, 0:BHC], SUB, MUL)
                    c1 = sb.tile([128, BHC], F32, name=f"{CX}c1_{tL1}",
                                 tag=f"{CX}c1", bufs=2)
                    if tL1 == 0:
                        nc.vector.tensor_scalar_mul(c1[:, :], ig1[:, :], 2.0)
                    else:
                        fc1 = sb.tile([128, BHC], F32, name=f"{CX}fc1_{tL1}",
                                      tag=f"{CX}fc1", bufs=2)
                        nc.vector.tensor_tensor(fc1[:, :], ga[:, 3 * BHC:4 * BHC],
                                                sX["c1"][:, :], MUL)
                        nc.vector.scalar_tensor_tensor(
                            c1[:, :], ig1[:, :], 2.0, fc1[:, :], MUL, ADD)
                    sX["c1"] = c1

            # ---- final linear head per chain (h2(SEQ-1) computed in-loop) ----
            for ch in range(NCH):
                s = st[ch]
                C = f"c{ch}_"
                Pf = ps.tile([2, BHC], F32, name=f"Pf{ch}", tag=f"{C}P0")
                nc.tensor.matmul(Pf[:, :], tfcw[:, :], s["h2"][:, :], start=True, stop=True)
                ob = sb.tile([2, BHC], F32, name=f"ob{ch}")
                nc.scalar.activation(ob[:, :], Pf[:, :], AF.Identity, bias=tfcb[:, 0:1])
                nc.sync.dma_start(out[ch, :, :], ob[:, :])

    nc.compile()
    return nc


def _prep_weights(w_ih0, w_hh0, b_ih0, b_hh0, w_ih1, w_hh1, b_ih1, b_hh1, fc_w, fc_b):
    """Host-side packing.  Device gate order: L0 [i, g, f, o], L1 [i, g, o, f]
    (PyTorch order is i, f, g, o)."""
    H = HIDDEN
    GATES0 = [0, 2, 1, 3]       # device k -> pytorch gate for L0 [i, g, f, o]
    GATES1 = [0, 2, 3, 1]       # device k -> pytorch gate for L1 [i, g, o, f]
    b0 = (b_ih0 + b_hh0).reshape(4, H)
    b1 = (b_ih1 + b_hh1).reshape(4, H)
    wi0 = w_ih0.reshape(4, H, INPUT)
    wh0_ = w_hh0.reshape(4, H, H)
    wi1 = w_ih1.reshape(4, H, H)
    wh1_ = w_hh1.reshape(4, H, H)

    wx0 = np.zeros((4, 2 * D1, 128), np.float32)
    wh0 = np.zeros((4, 128, 128), np.float32)
    w1a = np.zeros((4, 128, 128), np.float32)
    w1b = np.zeros((4, 128, 128), np.float32)
    b1st = np.zeros((4, 1, 128), np.float32)
    b1c = [None] * 4
    for k, gi in enumerate(GATES0):
        sc = 2.0 if gi == 2 else 1.0      # pytorch gate 2 = g: pre-scale x2
        wt = sc * wi0[gi].T               # [INPUT, H]
        wx0[k, :INPUT, 0:H] = wt
        wx0[k, INPUT, 0:H] = sc * b0[gi]
        wx0[k, D1:D1 + INPUT, H:2 * H] = wt
        wx0[k, D1 + INPUT, H:2 * H] = sc * b0[gi]
        wh0[k, 0:H, 0:H] = sc * wh0_[gi].T
        wh0[k, H:2 * H, H:2 * H] = sc * wh0_[gi].T
    for k, gi in enumerate(GATES1):
        sc = 2.0 if gi == 2 else 1.0
        w1a[k, 0:H, 0:H] = sc * wi1[gi].T
        w1a[k, H:2 * H, H:2 * H] = sc * wi1[gi].T
        w1b[k, 0:H, 0:H] = sc * wh1_[gi].T
        w1b[k, H:2 * H, H:2 * H] = sc * wh1_[gi].T
        b1st[k, 0, 0:H] = sc * b1[gi]
        b1st[k, 0, H:2 * H] = sc * b1[gi]
        if np.all(b1[gi] == b1[gi][0]):
            b1c[k] = float(sc * b1[gi][0])

    fcw = np.zeros((128, 2), np.float32)
    fcw[0:H, 0] = fc_w[0]
    fcw[H:2 * H, 1] = fc_w[0]
    fcb = np.full((2, 1), np.float32(fc_b[0]), np.float32)
    return wx0, wh0, w1a, w1b, b1st, b1c, fcw, fcb


def run_full(x, w_ih0, w_hh0, b_ih0, b_hh0, w_ih1, w_hh1, b_ih1, b_hh1, fc_w, fc_b,
             trace=False):
    """Run the full problem on 8 cores; returns (output [BATCH], BassKernelResults)."""
    from concourse.bass_utils import run_bass_kernel_spmd

    x = np.asarray(x, np.float32)
    args = [np.asarray(a, np.float32) for a in
            (w_ih0, w_hh0, b_ih0, b_hh0, w_ih1, w_hh1, b_ih1, b_hh1, fc_w, fc_b)]
    wx0, wh0, w1a, w1b, b1st, b1c, fcw, fcb = _prep_weights(*args)

    b1_const = b1c if all(v is not None for v in b1c) else None
    key = ("const", tuple(b1c)) if b1_const is not None else ("vec",)
    if key not in _CACHE:
        _CACHE.clear()
        _CACHE[key] = _build_module(b1_const)
    nc = _CACHE[key]

    bpat = np.zeros((1, 4 * BHC), np.float32)
    if b1_const is not None:
        for g in range(4):
            bpat[0, g * BHC:(g + 1) * BHC] = b1_const[g]

    in_maps = []
    for c in range(NCORES):
        xs = x[c * BCORE:(c + 1) * BCORE]                  # [BCORE, SEQ, INPUT]
        xT = np.empty((SEQ, NCH, 2 * D1, BHC), np.float32)
        for ch in range(NCH):
            a0 = ch * BHC
            b0_ = BCORE // 2 + ch * BHC
            xT[:, ch, :INPUT, :] = xs[a0:a0 + BHC].transpose(1, 2, 0)
            xT[:, ch, INPUT, :] = 1.0
            xT[:, ch, D1:D1 + INPUT, :] = xs[b0_:b0_ + BHC].transpose(1, 2, 0)
            xT[:, ch, D1 + INPUT, :] = 1.0
        in_maps.append({
            "xT": xT, "wx0": wx0, "wh0": wh0, "w1a": w1a, "w1b": w1b,
            "b1st": b1st, "bpat": bpat, "ones": np.ones((1, 4 * BHC), np.float32),
            "fcw": fcw, "fcb": fcb,
        })

    res = run_bass_kernel_spmd(nc, in_maps, core_ids=list(range(NCORES)), trace=trace)
    outs = []
    for r in res.results:
        o = r["out"]                        # [NCH, 2, BHC]: (chain, half, col)
        # per-core batch order: [ch0 halfA, ch1 halfA, ch0 halfB, ch1 halfB]
        outs.append(o.transpose(1, 0, 2).reshape(BCORE))
    return np.concatenate(outs, axis=0).astype(np.float32), res


def kernel(x, w_ih0, w_hh0, b_ih0, b_hh0, w_ih1, w_hh1, b_ih1, b_hh1, fc_w, fc_b):
    out, _ = run_full(x, w_ih0, w_hh0, b_ih0, b_hh0,
                      w_ih1, w_hh1, b_ih1, b_hh1, fc_w, fc_b)
    return out
